# revision 1
# baseline (speedup 1.0000x reference)
"""Causal self-attention Trainium2 kernel.

Problem: B=2, L=2048, D=1024, 16 heads (hd=64), fp32.

Sharding (8 cores): core = (batch b in {0,1}) x (head-group g in {0..3} of 4
heads). Each core:
  - reads x[b]^T  [1024, 2048]
  - QKV projection for its 4 heads (fp32r matmuls, full PE rate at N>=256)
  - causal attention in transposed layout:
      S^T[k, q] = K^T(lhsT) x Q^T(rhs), two heads row-packed per matmul (K=64)
      P^T = exp(S^T)  (ACT), causal mask via 0/1 mask multiply (DVE)
      O^T[d, q] accumulated as [V | ones]^T(lhsT) x P^T(rhs) -> row 64 = rowsum
      normalize: copy frees PSUM (DVE); recip = exp(-ln(rowsum)) on ACT;
      partition-broadcast via K=1 PE matmul; final multiply on DVE
  - output projection partial: OUT[tok, :] = O^T-chunks(lhsT) x Wo^T(rhs)
Host: sums the 4 head-group partials per batch, adds out_b.

All matmul operands are typed float32r (TF32-like, ~1.5e-4 rel err/matmul,
full 1 cycle/row PE rate at N>=256 vs 4 cycles/row for fp32).
"""
import os
import numpy as np

import concourse.bass as bass
import concourse.mybir as mybir
import concourse.tile as tile
from concourse import bacc
from concourse.bass_utils import run_bass_kernel_spmd

F32 = mybir.dt.float32
F32R = mybir.dt.float32r
AF = mybir.ActivationFunctionType

D_MODEL = 1024
N_HEADS = 16
HD = 64
B = 2
L = 2048                      # tokens per batch
HPC = 4                       # heads per core
DG = HPC * HD                 # 256 dims per core's head group
QB = 512                      # q-block width
N_QB = L // QB                # 4
N_KC = L // 128               # 16 k-chunks of 128 tokens
N_DC = D_MODEL // 128         # 8 d_model chunks
N_TT = L // 128               # 16 token tiles


def _patch_act_tables():
    """Make Exp and Ln resolve to the one table set containing both, so the
    per-qblock reciprocal (exp(-ln(x))) never triggers a ~2.7us table switch."""
    from concourse.hw_specs import get_activation_tables
    tabs = get_activation_tables("gen3")
    combined = "natural_log_exp_and_others"
    if combined in tabs:
        for name, fns in tabs.items():
            if name != combined:
                fns.discard(AF.Exp)
                fns.discard(AF.Ln)


def _build():
    _patch_act_tables()
    nc = bacc.Bacc("TRN2", target_bir_lowering=False)

    xt = nc.dram_tensor("xt", [N_QB, 128, N_DC, QB], F32R, kind="ExternalInput")
    wq = nc.dram_tensor("wq", [128, N_DC, DG], F32R, kind="ExternalInput")
    wk = nc.dram_tensor("wk", [128, N_DC, DG], F32R, kind="ExternalInput")
    wv = nc.dram_tensor("wv", [128, N_DC, DG], F32R, kind="ExternalInput")
    wo = nc.dram_tensor("wo", [128, 2, D_MODEL], F32R, kind="ExternalInput")
    bq = nc.dram_tensor("bq", [128, 2], F32, kind="ExternalInput")
    bk = nc.dram_tensor("bk", [128, 2], F32, kind="ExternalInput")
    bv = nc.dram_tensor("bv", [1, DG], F32R, kind="ExternalInput")
    # masks[p, i, c, q] = 1 iff -256*i + q - 128*c - p >= 0  (i in {0,1})
    masks = nc.dram_tensor("masks", [128, 2, 2, QB], F32R, kind="ExternalInput")
    out = nc.dram_tensor("out", [L, D_MODEL], F32, kind="ExternalOutput")

    with tile.TileContext(nc) as tc:
        with (
            tc.tile_pool(name="cst", bufs=1) as cst,
            tc.tile_pool(name="xtp", bufs=3) as xtp,
            tc.tile_pool(name="ptp", bufs=3) as ptp,
            tc.tile_pool(name="nrm", bufs=3) as nrm,
            tc.tile_pool(name="osb", bufs=2) as osb,
            tc.tile_pool(name="ps_st", bufs=1, space="PSUM") as ps_st,
            tc.tile_pool(name="ps_ot", bufs=1, space="PSUM") as ps_ot,
            tc.tile_pool(name="ps_mm", bufs=2, space="PSUM") as ps_mm,
        ):
            # ---- constants / weights ----
            wq_sb = cst.tile([128, N_DC, DG], F32R, tag="wq")
            wk_sb = cst.tile([128, N_DC, DG], F32R, tag="wk")
            wv_sb = cst.tile([128, N_DC, DG], F32R, tag="wv")
            wo_sb = cst.tile([128, 2, D_MODEL], F32R, tag="wo")
            bq_sb = cst.tile([128, 2], F32, tag="bq")
            bk_sb = cst.tile([128, 2], F32, tag="bk")
            bv_sb = cst.tile([1, DG], F32R, tag="bv")
            mask_sb = cst.tile([128, 2, 2, QB], F32R, tag="mask")
            # DMA queues: SP ring carries xt blocks + outputs (critical
            # path), SWDGE (gpsimd) carries weights/masks; ACT ring stays free
            nc.sync.dma_start(wq_sb[:, 0:2], wq[:, 0:2, :])
            nc.sync.dma_start(bq_sb, bq[:, :])

            ones_f = cst.tile([128, HPC], F32, tag="ones_f")
            nc.vector.memset(ones_f, 1.0)
            ones1_f = cst.tile([1, 128], F32, tag="ones1_f")
            nc.vector.memset(ones1_f, 1.0)
            ones1 = cst.tile([1, 128], F32R, tag="ones1")
            nc.vector.tensor_copy(ones1, ones1_f)

            # ---- resident activation tensors ----
            # QT/KT: per head-pair t: [128 (2x64 dims), L]
            qt_sb = [cst.tile([128, L], F32R, tag=f"qt{t}", name=f"qt{t}")
                     for t in range(2)]
            kt_sb = [cst.tile([128, L], F32R, tag=f"kt{t}", name=f"kt{t}")
                     for t in range(2)]
            # OT: per head-pair t: [128 (2x64 dims), L] (normalized)
            ot_sb = [cst.tile([128, L], F32R, tag=f"ot{t}", name=f"ot{t}")
                     for t in range(2)]
            # V natural with ones column: per token tile: [128 tok, 4 heads, 65]
            v_sb = [cst.tile([128, HPC, HD + 1], F32R, tag=f"v{tt}", name=f"v{tt}")
                    for tt in range(N_TT)]

            def load_xt(tb, fine=False):
                xt_t = xtp.tile([128, N_DC, QB], F32R, tag="xt", name="xt_t")
                if fine:
                    nc.sync.dma_start(xt_t[:, 0:2], xt[tb, :, 0:2])
                    nc.sync.dma_start(wq_sb[:, 2:4], wq[:, 2:4, :])
                    nc.sync.dma_start(xt_t[:, 2:4], xt[tb, :, 2:4])
                    nc.sync.dma_start(xt_t[:, 4:], xt[tb, :, 4:])
                else:
                    nc.sync.dma_start(xt_t[:, 0:N_DC // 2], xt[tb, :, 0:N_DC // 2])
                    nc.sync.dma_start(xt_t[:, N_DC // 2:], xt[tb, :, N_DC // 2:])
                return xt_t

            def qkv_units(tb, xt_t):
                """QKV projection for token block tb as a list of thunks."""
                units = []

                def qk_unit(t, w_sb, b_sb, dst):
                    def f():
                        acc = ps_mm.tile([128, QB], F32, tag="mm", name="acc")
                        for c in range(N_DC):
                            nc.tensor.matmul(
                                acc,
                                w_sb[:, c, 128 * t:128 * (t + 1)],
                                xt_t[:, c, :],
                                start=(c == 0), stop=(c == N_DC - 1),
                            )
                        nc.vector.tensor_scalar_add(
                            dst[t][:, tb * QB:(tb + 1) * QB], acc, b_sb[:, t:t + 1],
                        )
                    return f

                def v_unit(j):
                    def f():
                        tt = tb * (QB // 128) + j
                        vps = ps_mm.tile([128, DG], F32, tag="mm", name="vps")
                        for c in range(N_DC):
                            nc.tensor.matmul(
                                vps,
                                xt_t[:, c, j * 128:(j + 1) * 128],
                                wv_sb[:, c, :],
                                start=(c == 0), stop=False,
                            )
                        nc.tensor.matmul(vps, ones1, bv_sb, start=False, stop=True)
                        nc.vector.tensor_copy(
                            v_sb[tt][:, :, 0:HD],
                            vps.rearrange("p (h d) -> p h d", h=HPC),
                        )
                        nc.vector.tensor_copy(v_sb[tt][:, :, HD], ones_f)
                    return f

                for t in range(2):
                    units.append(qk_unit(t, wq_sb, bq_sb, qt_sb))
                    units.append(qk_unit(t, wk_sb, bk_sb, kt_sb))
                for j in range(QB // 128):
                    units.append(v_unit(j))
                return units

            def attn_units(qb):
                """Attention for q-block qb as a list of thunks.

                Units stay ordered per head-pair t (ot_p accumulates across
                groups); the norm unit of pair t frees its PSUM banks.
                """
                n_kc = 4 * (qb + 1)
                n_g = n_kc // 2
                units = []
                ot_state = {}

                def group_unit(t, g):
                    def f():
                        if g == 0:
                            ot_state[t] = {
                                hp: ps_ot.tile([HD + 1, QB], F32, tag=f"otp{hp}",
                                               name=f"otp{hp}") for hp in range(2)}
                        ot_p = ot_state[t]
                        st = [ps_st.tile([128, 2, QB], F32, tag=f"st{hp}",
                                         name=f"st{hp}") for hp in range(2)]
                        for c in range(2):
                            kc = 2 * g + c
                            for hp in range(2):
                                nc.tensor.matmul(
                                    st[hp][:, c, :],
                                    kt_sb[t][64 * hp:64 * (hp + 1),
                                             kc * 128:(kc + 1) * 128],
                                    qt_sb[t][64 * hp:64 * (hp + 1),
                                             qb * QB:(qb + 1) * QB],
                                    start=True, stop=True,
                                )
                        base = QB * qb - 256 * g
                        for hp in range(2):
                            p_t = ptp.tile([128, 2, QB], F32R, tag=f"pt{hp}",
                                           name=f"pt{hp}")
                            nc.scalar.activation(p_t, st[hp], AF.Exp)
                            if base < 255:   # diagonal group
                                mi = (-base) // 256
                                nc.vector.tensor_mul(p_t, p_t, mask_sb[:, mi, :, :])
                            for c in range(2):
                                kc = 2 * g + c
                                nc.tensor.matmul(
                                    ot_p[hp],
                                    v_sb[kc][:, 2 * t + hp, 0:HD + 1],
                                    p_t[:, c, :],
                                    start=(kc == 0), stop=(kc == n_kc - 1),
                                )
                    return f

                def norm_unit(t):
                    def f():
                        ot_p = ot_state[t]
                        for hp in range(2):
                            ot_un = nrm.tile([HD + 1, QB], F32, tag="ot_un",
                                             name="ot_un")
                            nc.vector.tensor_copy(ot_un, ot_p[hp])
                            lnrs = nrm.tile([1, QB], F32, tag="lnrs", name="lnrs")
                            nc.scalar.activation(lnrs, ot_un[HD:HD + 1, :], AF.Ln)
                            recip = nrm.tile([1, QB], F32R, tag="recip",
                                             name="recip")
                            nc.scalar.activation(recip, lnrs, AF.Exp, scale=-1.0)
                            # broadcast across partitions via K=1 PE matmul
                            rbc = ps_mm.tile([64, QB], F32, tag="mm", name="rbc")
                            nc.tensor.matmul(rbc, ones1[:, 0:HD], recip,
                                             start=True, stop=True)
                            nc.vector.tensor_mul(
                                ot_sb[t][64 * hp:64 * (hp + 1),
                                         qb * QB:(qb + 1) * QB],
                                ot_un[0:HD, :],
                                rbc,
                            )
                    return f

                for t in range(2):
                    for g in range(n_g):
                        units.append(group_unit(t, g))
                    units.append(norm_unit(t))
                return units

            def outproj_units(qb):
                units = []

                def op_unit(j):
                    def f():
                        tt = qb * (QB // 128) + j
                        ob = osb.tile([128, D_MODEL], F32, tag="ob", name="ob")
                        for dc in range(2):
                            ops = ps_mm.tile([128, 512], F32, tag="mm", name="ops")
                            for t in range(2):
                                nc.tensor.matmul(
                                    ops,
                                    ot_sb[t][:, tt * 128:(tt + 1) * 128],
                                    wo_sb[:, t, dc * 512:(dc + 1) * 512],
                                    start=(t == 0), stop=(t == 1),
                                )
                            nc.vector.tensor_copy(
                                ob[:, dc * 512:(dc + 1) * 512], ops)
                        nc.sync.dma_start(out[tt * 128:(tt + 1) * 128, :], ob)
                    return f

                for j in range(QB // 128):
                    units.append(op_unit(j))
                return units

            def emit_interleaved(a_units, b_units):
                """Merge two unit lists proportionally (a is the longer/primary
                stream); keeps relative order within each list."""
                na, nb = len(a_units), len(b_units)
                if nb == 0:
                    for u in a_units:
                        u()
                    return
                bi = 0
                for ai, u in enumerate(a_units):
                    u()
                    # after unit ai, emit floor((ai+1)*nb/na) - bi b-units
                    want = ((ai + 1) * nb) // na
                    while bi < want:
                        b_units[bi]()
                        bi += 1
                while bi < nb:
                    b_units[bi]()
                    bi += 1

            # ---- emission: interleave attention with QKV/outproj so the PE
            # stays dense (HAM stays un-throttled) while ACT chews the exps ----
            xt0 = load_xt(0, fine=True)
            nc.sync.dma_start(wq_sb[:, N_DC // 2:], wq[:, N_DC // 2:, :])
            nc.sync.dma_start(wk_sb, wk[:, :, :])
            nc.sync.dma_start(bk_sb, bk[:, :])
            nc.sync.dma_start(wv_sb, wv[:, :, :])
            nc.sync.dma_start(bv_sb, bv[:, :])
            xt1 = load_xt(1)
            nc.gpsimd.dma_start(mask_sb, masks[:, :, :, :])
            nc.gpsimd.dma_start(wo_sb, wo[:, :, :])
            for u in qkv_units(0, xt0):
                u()
            emit_interleaved(attn_units(0), qkv_units(1, xt1))
            xt2 = load_xt(2)
            xt3 = load_xt(3)
            emit_interleaved(attn_units(1), qkv_units(2, xt2) + outproj_units(0))
            emit_interleaved(attn_units(2), qkv_units(3, xt3) + outproj_units(1))
            emit_interleaved(attn_units(3), outproj_units(2))
            for u in outproj_units(3):
                u()

    nc.compile()
    return nc


_NC_CACHE = None


def _get_nc():
    global _NC_CACHE
    if _NC_CACHE is None:
        _NC_CACHE = _build()
    return _NC_CACHE


def _sw_w(w):
    """[C*128, M] -> [128, C, M] (SBUF layout, contiguous per partition)."""
    c128, m = w.shape
    return np.ascontiguousarray(w.reshape(c128 // 128, 128, m).transpose(1, 0, 2))


def _make_masks():
    p_ = np.arange(128)[:, None, None, None]
    i_ = np.arange(2)[None, :, None, None]
    c_ = np.arange(2)[None, None, :, None]
    q_ = np.arange(QB)[None, None, None, :]
    return np.ascontiguousarray(
        ((-256 * i_ + q_ - 128 * c_ - p_) >= 0).astype(np.float32))


def kernel(x, qkv_w, qkv_b, out_w, out_b, _trace=False):
    x = np.asarray(x, dtype=np.float32)
    qkv_w = np.asarray(qkv_w, dtype=np.float32)
    qkv_b = np.asarray(qkv_b, dtype=np.float32)
    out_w = np.asarray(out_w, dtype=np.float32)
    out_b = np.asarray(out_b, dtype=np.float32)

    scale = 1.0 / np.sqrt(HD)
    wq_full = qkv_w[0:D_MODEL] * scale          # [1024, 1024]
    wk_full = qkv_w[D_MODEL:2 * D_MODEL]
    wv_full = qkv_w[2 * D_MODEL:3 * D_MODEL]
    bq_full = qkv_b[0:D_MODEL] * scale
    bk_full = qkv_b[D_MODEL:2 * D_MODEL]
    bv_full = qkv_b[2 * D_MODEL:3 * D_MODEL]

    masks = _make_masks()
    in_maps = []
    for core in range(8):
        b, g = core // 4, core % 4
        sl = slice(DG * g, DG * (g + 1))
        # xt: x[b]^T [1024, 2048] -> [qb, p, c, t] = [4, 128, 8, 512]
        xt_sw = np.ascontiguousarray(
            x[b].T.reshape(N_DC, 128, N_QB, QB).transpose(2, 1, 0, 3))
        in_maps.append({
            "xt": xt_sw,
            "wq": _sw_w(wq_full[sl].T),
            "wk": _sw_w(wk_full[sl].T),
            "wv": _sw_w(wv_full[sl].T),
            "wo": _sw_w(out_w[:, sl].T),
            "bq": np.ascontiguousarray(bq_full[sl].reshape(2, 128).T),
            "bk": np.ascontiguousarray(bk_full[sl].reshape(2, 128).T),
            "bv": np.ascontiguousarray(bv_full[sl].reshape(1, DG)),
            "masks": masks,
        })

    nc = _get_nc()
    res = run_bass_kernel_spmd(nc, in_maps, core_ids=list(range(8)),
                               trace=_trace)

    final = np.zeros((B, L, D_MODEL), dtype=np.float32)
    for core in range(8):
        b = core // 4
        final[b] += res.results[core]["out"]
    final += out_b[None, None, :]

    kernel.last_results = res
    return final

